# revision 75
# baseline (speedup 1.0000x reference)
"""MoE regressor (E=16, H=1024, B=4096, top-2) on 8 trn2 NeuronCores.

Expert-parallel with load-balanced pairing: experts are sorted by routed
token count and core i gets the i-th largest ("slot 0") plus the i-th
smallest ("slot 1") expert, so per-core work is near-uniform. The host
computes fp32 top-2 routing, gathers each expert's tokens and transposes
them to [H, C] layout in bf16; each core runs its two experts' 2-layer
MLP (bf16 matmuls, fp32 PSUM accumulation) and returns per-slot scalar
outputs; the host applies the softmax combine weights (scatter-add).

Self-contained: hardcodes all shapes.
"""

import numpy as np
import ml_dtypes

import concourse.bass as bass  # noqa: F401
from concourse import bacc
import concourse.mybir as mybir
import concourse.tile as tile
from concourse.bass_utils import run_bass_kernel_spmd

P = 128
B = 4096
H = 1024
E = 16
NCORES = 8
SLOTS = E // NCORES  # experts per core = 2

F32 = mybir.dt.float32
BF16 = mybir.dt.bfloat16
BF = ml_dtypes.bfloat16

_CACHE = {}


def _chunks(c0, c1):
    """Per-slot (offset, length) column chunks into the CT axis.

    Each chunk must fit one PSUM bank (<=512 fp32), so each slot is split
    into two near-equal chunks.
    """
    out = []
    for off, cs in ((0, c0), (c0, c1)):
        ca = (cs // 2 + 3) // 4 * 4
        out.append(((off, ca), (off + ca, cs - ca)))
    return out


def _build(c0, c1):
    ct = c0 + c1
    chunks = _chunks(c0, c1)
    nc = bacc.Bacc(None, target_bir_lowering=False)

    # ge chunks: [kp, k, c] bf16, transposed token embeddings
    g_dram = {}
    for s in range(SLOTS):
        for ci, (off, ln) in enumerate(chunks[s]):
            g_dram[(s, ci)] = nc.dram_tensor(
                f"g{s}{ci}", (P, 8, ln), BF16, kind="ExternalInput"
            )
    # w1p[s, m, kp, k, mp]: stationary tiles for layer 1
    w1p = nc.dram_tensor("w1p", (SLOTS, 8, P, 8, P), BF16, kind="ExternalInput")
    b1p = nc.dram_tensor("b1p", (P, 8, SLOTS), F32, kind="ExternalInput")
    # w2 columns replicated 4x: a 4-partition-wide L2 matmul output runs at
    # full PE rate where a 1-partition output runs at ~60%.
    w2p = nc.dram_tensor("w2p", (P, 8, SLOTS * 4), BF16, kind="ExternalInput")
    b2p = nc.dram_tensor("b2p", (1, SLOTS), F32, kind="ExternalInput")
    out = nc.dram_tensor("out", (1, ct), F32, kind="ExternalOutput")

    with tile.TileContext(nc) as tc:
        with (
            tc.tile_pool(name="const", bufs=1) as cpool,
            tc.tile_pool(name="ge", bufs=1) as gepool,
            tc.tile_pool(name="wp", bufs=4) as wpool,
            tc.tile_pool(name="hp", bufs=6) as hpool,
            tc.tile_pool(name="op", bufs=1) as opool,
            tc.tile_pool(name="ps1", bufs=2, space="PSUM") as ps1,
            tc.tile_pool(name="ps2", bufs=1, space="PSUM") as ps2,
            tc.tile_pool(name="psw", bufs=1, space="PSUM") as psw,
        ):
            g_sb = {}
            for s in range(SLOTS):
                for ci, (off, ln) in enumerate(chunks[s]):
                    g_sb[(s, ci)] = gepool.tile(
                        [P, 8, ln], BF16, tag=f"g{s}{ci}", name=f"g{s}{ci}"
                    )
            # B_DELAY > 0 defers slot-0 chunk B's first m-sweeps past m=7
            # for an earlier compute start; measured net-negative at every
            # setting — the single-chunk phase burns w1 tiles and ge
            # bandwidth faster than the DMA pipeline (ring depth 4, ~4.3us
            # latency, ge chunks serialize at ~3us each) can deliver.
            B_DELAY = 0

            sm_order = [(s, m) for s in range(SLOTS) for m in range(8)]
            w1_tiles = {}

            def fetch_w1(i):
                if i >= len(sm_order):
                    return
                s_, m_ = sm_order[i]
                if s_ == 0 and m_ < B_DELAY:
                    # reused by the deferred chunk-B sweep at slot end
                    t = wpool.tile(
                        [P, 8, P], BF16, tag=f"w1k{m_}", name="w1k", bufs=1
                    )
                else:
                    t = wpool.tile([P, 8, P], BF16, tag="w1t", name="w1t")
                nc.sync.dma_start(t, w1p[s_, m_])
                w1_tiles[(s_, m_)] = t

            # Transfer completion is roughly dispatch_time + ~4.3us fixed
            # latency, so dispatch-queue position is everything. The two
            # dispatch queues run in parallel: Scalar takes ge chunks +
            # consts ordered by first need; Sync takes the whole w1 stream.
            b1_sb = cpool.tile([P, 8, SLOTS], F32)
            w2_sb = cpool.tile([P, 8, SLOTS * 4], BF16)
            b2_sb = cpool.tile([1, SLOTS], F32)
            with tc.high_priority():
                nc.scalar.dma_start(g_sb[(0, 0)], g_dram[(0, 0)][:, :, :])
                nc.scalar.dma_start(g_sb[(0, 1)], g_dram[(0, 1)][:, :, :])
                nc.scalar.dma_start(g_sb[(1, 0)], g_dram[(1, 0)][:, :, :])
                nc.scalar.dma_start(g_sb[(1, 1)], g_dram[(1, 1)][:, :, :])
                fetch_w1(0)
                fetch_w1(1)
                nc.sync.dma_start(b1_sb, b1p[:, :, :])
                nc.sync.dma_start(w2_sb, w2p[:, :, :])
                nc.sync.dma_start(b2_sb, b2p[:, :])

            # PE warmup: burn the p-state ramp on dummy matmuls while the
            # first ge/w1 DMAs are in flight.
            warm_sb = cpool.tile([P, 512], BF16)
            nc.vector.memset(warm_sb, 0.0)
            pwarm = psw.tile([P, 512], F32)
            for _ in range(13):
                nc.tensor.matmul(
                    pwarm, warm_sb[:, :P], warm_sb, start=True, stop=True
                )

            out_sb = opool.tile([1, ct], F32)

            # Batch schedule: each batch is (s, fetch_i, [(ci, m, first,
            # last)]) — `first`/`last` bracket that chunk's L1/L2 PSUM
            # accumulation group. Slot 0 defers chunk B by B_DELAY m-steps.
            sched = []
            fu = 2  # w1 prefetch pointer (2 already issued pre-loop)
            for s in range(SLOTS):
                bd = B_DELAY if s == 0 else 0
                for m in range(8):
                    ent = [(0, m, m == 0, m == 7)]
                    if m >= bd:
                        ent.append((1, m, m == bd, bd == 0 and m == 7))
                    sched.append((s, fu, ent))
                    fu += 1
                for m in range(bd):
                    sched.append(
                        (s, None, [(1, m, bd == 8 and m == 0, m == bd - 1)])
                    )

            pending = []  # [(s, p2s, [(ci, m, h, first, last)])]
            p2cur = None
            cur_s = None

            def emit_l2(pend):
                """Second-layer matmuls one batch behind, so the PE never
                waits on the Scalar engine's ReLU."""
                s_, p2s_, ents = pend
                for ci, m_, hsb, first, last in ents:
                    nc.tensor.matmul(
                        p2s_[ci], w2_sb[:, m_, s_ * 4:s_ * 4 + 4], hsb,
                        start=first, stop=last,
                    )
                    if last:
                        off, ln = chunks[s_][ci]
                        nc.vector.tensor_scalar_add(
                            out_sb[:, off:off + ln], p2s_[ci][0:1, :],
                            b2_sb[:, s_:s_ + 1],
                        )
                        nc.sync.dma_start(
                            out[:, off:off + ln], out_sb[:, off:off + ln]
                        )

            for s, fetch_i, ents in sched:
                if s != cur_s:
                    cur_s = s
                    p2cur = [
                        ps2.tile([4, ln], F32, tag=f"p2_{ci}", name=f"p2_{ci}")
                        for ci, (off, ln) in enumerate(chunks[s])
                    ]
                if fetch_i is not None and fetch_i < 16:
                    fetch_w1(fetch_i)
                m = ents[0][1]
                w1t = w1_tiles[(s, m)]
                p1s = {}
                for ci, m_, first, last in ents:
                    ln = chunks[s][ci][1]
                    p1s[ci] = ps1.tile(
                        [P, ln], F32, tag=f"p1_{ci}", name=f"p1_{ci}"
                    )
                for k in range(8):
                    for ci, m_, first, last in ents:
                        nc.tensor.matmul(
                            p1s[ci], w1t[:, k], g_sb[(s, ci)][:, k, :],
                            start=(k == 0), stop=(k == 7),
                        )
                # flush L2 in pairs: batching two m-steps' L2 matmuls into
                # one stationary-switch window halves the w1<->w2 weight
                # buffer switches (each costs a ~90ns pipeline bubble)
                if len(pending) == 4:
                    for pend in pending:
                        emit_l2(pend)
                    pending = []
                pents = []
                for ci, m_, first, last in ents:
                    ln = chunks[s][ci][1]
                    hsb = hpool.tile([P, ln], BF16, tag=f"h_{ci}")
                    nc.scalar.activation(
                        hsb, p1s[ci],
                        mybir.ActivationFunctionType.Relu,
                        bias=b1_sb[:, m_, s:s + 1],
                    )
                    pents.append((ci, m_, hsb, first, last))
                pending.append((s, p2cur, pents))
            for pend in pending:
                emit_l2(pend)
    nc.finalize()
    return nc


def _route_host(emb, rw, rb):
    logits = emb.astype(np.float32) @ rw.astype(np.float32) + rb.astype(np.float32)
    i1 = np.argmax(logits, axis=1)
    l1 = logits[np.arange(B), i1]
    l2m = logits.copy()
    l2m[np.arange(B), i1] = -np.inf
    i2 = np.argmax(l2m, axis=1)
    l2 = l2m[np.arange(B), i2]
    d = np.exp(l2 - l1)
    w1 = (1.0 / (1.0 + d)).astype(np.float32)
    w2 = (1.0 - w1).astype(np.float32)
    comb = np.zeros((B, E), np.float32)
    comb[np.arange(B), i1] = w1
    comb[np.arange(B), i2] = w2
    return comb


def kernel(embeddings, router_w, router_b, w1, b1, w2, b2):
    emb = np.ascontiguousarray(np.asarray(embeddings, dtype=np.float32))
    rw = np.asarray(router_w, np.float32)
    rb = np.asarray(router_b, np.float32)
    w1 = np.asarray(w1, np.float32)
    b1 = np.asarray(b1, np.float32)
    w2 = np.asarray(w2, np.float32)
    b2 = np.asarray(b2, np.float32)

    comb = _route_host(emb, rw, rb)
    counts = (comb > 0).sum(axis=0)

    # Balanced pairing: i-th largest with i-th smallest expert per core.
    order = np.argsort(counts)
    slot_experts = [  # [slot][core] -> expert id
        [int(order[E - 1 - c]) for c in range(NCORES)],
        [int(order[c]) for c in range(NCORES)],
    ]
    pad = lambda n: max(8, -(-int(n) // 4) * 4)
    c0 = pad(max(counts[e] for e in slot_experts[0]))
    c1 = pad(max(counts[e] for e in slot_experts[1]))
    ct = c0 + c1
    key = (c0, c1)
    if key not in _CACHE:
        _CACHE[key] = _build(c0, c1)
    nc = _CACHE[key]
    chunks = _chunks(c0, c1)

    emb_bf = emb.astype(BF)
    in_maps = []
    toks = []
    for c in range(NCORES):
        m = {}
        ctoks = []
        for s, cs in ((0, c0), (1, c1)):
            e = slot_experts[s][c]
            ids = np.nonzero(comb[:, e] > 0)[0]
            ctoks.append(ids)
            geT = np.zeros((P, 8, cs), BF)
            n = len(ids)
            # [n, 1024] -> [128(kp), 8(k), n]
            geT[:, :, :n] = emb_bf[ids].reshape(n, 8, P).transpose(2, 1, 0)
            for ci, (off, ln) in enumerate(chunks[s]):
                rel = off - (0 if s == 0 else c0)
                m[f"g{s}{ci}"] = np.ascontiguousarray(geT[:, :, rel:rel + ln])
        es = [slot_experts[s][c] for s in range(SLOTS)]
        # w1[e]: [h_in(k,kp), h_out(m,mp)] -> [m, kp, k, mp]
        m["w1p"] = np.ascontiguousarray(
            w1[es].reshape(SLOTS, 8, P, 8, P).transpose(0, 3, 2, 1, 4)
        ).astype(BF)
        m["b1p"] = np.ascontiguousarray(
            b1[es].reshape(SLOTS, 8, P).transpose(2, 1, 0)
        )
        m["w2p"] = np.ascontiguousarray(
            np.repeat(
                w2[es, :, 0].reshape(SLOTS, 8, P).transpose(2, 1, 0), 4, axis=2
            )
        ).astype(BF)
        m["b2p"] = np.ascontiguousarray(b2[es, 0].reshape(1, SLOTS))
        toks.append(ctoks)
        in_maps.append(m)

    res = run_bass_kernel_spmd(nc, in_maps, core_ids=list(range(NCORES)))

    out = np.zeros((B,), np.float32)
    for c in range(NCORES):
        o = res.results[c]["out"][0]  # [ct]
        for s, off in ((0, 0), (1, c0)):
            e = slot_experts[s][c]
            ids = toks[c][s]
            out[ids] += comb[ids, e] * o[off:off + len(ids)]
    return out.reshape(B, 1)


# revision 77
# speedup vs baseline: 1.0348x; 1.0348x over previous
"""MoE regressor (E=16, H=1024, B=4096, top-2) on 8 trn2 NeuronCores.

Expert-parallel with load-balanced pairing: experts are sorted by routed
token count and core i gets the i-th largest ("slot 0") plus the i-th
smallest ("slot 1") expert, so per-core work is near-uniform. The host
computes fp32 top-2 routing, gathers each expert's tokens and transposes
them to [H, C] layout in bf16; each core runs its two experts' 2-layer
MLP (bf16 matmuls, fp32 PSUM accumulation) and returns per-slot scalar
outputs; the host applies the softmax combine weights (scatter-add).

Self-contained: hardcodes all shapes.
"""

import numpy as np
import ml_dtypes

import concourse.bass as bass  # noqa: F401
from concourse import bacc
import concourse.mybir as mybir
import concourse.tile as tile
from concourse.bass_utils import run_bass_kernel_spmd

P = 128
B = 4096
H = 1024
E = 16
NCORES = 8
SLOTS = E // NCORES  # experts per core = 2

F32 = mybir.dt.float32
BF16 = mybir.dt.bfloat16
BF = ml_dtypes.bfloat16

_CACHE = {}


def _chunks(c0, c1):
    """Per-slot (offset, length) column chunks into the CT axis.

    Each chunk must fit one PSUM bank (<=512 fp32), so each slot is split
    into two near-equal chunks.
    """
    out = []
    for off, cs in ((0, c0), (c0, c1)):
        ca = (cs // 2 + 3) // 4 * 4
        out.append(((off, ca), (off + ca, cs - ca)))
    return out


def _build(c0, c1):
    ct = c0 + c1
    chunks = _chunks(c0, c1)
    nc = bacc.Bacc(None, target_bir_lowering=False)

    # ge chunks: [kp, k, c] bf16, transposed token embeddings
    g_dram = {}
    for s in range(SLOTS):
        for ci, (off, ln) in enumerate(chunks[s]):
            g_dram[(s, ci)] = nc.dram_tensor(
                f"g{s}{ci}", (P, 8, ln), BF16, kind="ExternalInput"
            )
    # w1p[s, m, kp, k, mp]: stationary tiles for layer 1
    w1p = nc.dram_tensor("w1p", (SLOTS, 8, P, 8, P), BF16, kind="ExternalInput")
    b1p = nc.dram_tensor("b1p", (P, 8, SLOTS), F32, kind="ExternalInput")
    # w2 columns replicated 4x: a 4-partition-wide L2 matmul output runs at
    # full PE rate where a 1-partition output runs at ~60%.
    w2p = nc.dram_tensor("w2p", (P, 8, SLOTS * 4), BF16, kind="ExternalInput")
    b2p = nc.dram_tensor("b2p", (1, SLOTS), F32, kind="ExternalInput")
    out = nc.dram_tensor("out", (1, ct), F32, kind="ExternalOutput")

    with tile.TileContext(nc) as tc:
        with (
            tc.tile_pool(name="const", bufs=1) as cpool,
            tc.tile_pool(name="ge", bufs=1) as gepool,
            tc.tile_pool(name="wp", bufs=4) as wpool,
            tc.tile_pool(name="hp", bufs=10) as hpool,
            tc.tile_pool(name="op", bufs=1) as opool,
            tc.tile_pool(name="ps1", bufs=2, space="PSUM") as ps1,
            tc.tile_pool(name="ps2", bufs=1, space="PSUM") as ps2,
            tc.tile_pool(name="psw", bufs=1, space="PSUM") as psw,
        ):
            g_sb = {}
            for s in range(SLOTS):
                for ci, (off, ln) in enumerate(chunks[s]):
                    g_sb[(s, ci)] = gepool.tile(
                        [P, 8, ln], BF16, tag=f"g{s}{ci}", name=f"g{s}{ci}"
                    )
            # B_DELAY > 0 defers slot-0 chunk B's first m-sweeps past m=7
            # for an earlier compute start; measured net-negative at every
            # setting — the single-chunk phase burns w1 tiles and ge
            # bandwidth faster than the DMA pipeline (ring depth 4, ~4.3us
            # latency, ge chunks serialize at ~3us each) can deliver.
            B_DELAY = 0

            sm_order = [(s, m) for s in range(SLOTS) for m in range(8)]
            w1_tiles = {}

            def fetch_w1(i):
                if i >= len(sm_order):
                    return
                s_, m_ = sm_order[i]
                if s_ == 0 and m_ < B_DELAY:
                    # reused by the deferred chunk-B sweep at slot end
                    t = wpool.tile(
                        [P, 8, P], BF16, tag=f"w1k{m_}", name="w1k", bufs=1
                    )
                else:
                    t = wpool.tile([P, 8, P], BF16, tag="w1t", name="w1t")
                nc.sync.dma_start(t, w1p[s_, m_])
                w1_tiles[(s_, m_)] = t

            # Transfer completion is roughly dispatch_time + ~4.3us fixed
            # latency, so dispatch-queue position is everything. The two
            # dispatch queues run in parallel: Scalar takes ge chunks +
            # consts ordered by first need; Sync takes the whole w1 stream.
            b1_sb = cpool.tile([P, 8, SLOTS], F32)
            w2_sb = cpool.tile([P, 8, SLOTS * 4], BF16)
            b2_sb = cpool.tile([1, SLOTS], F32)
            with tc.high_priority():
                nc.scalar.dma_start(g_sb[(0, 0)], g_dram[(0, 0)][:, :, :])
                nc.scalar.dma_start(g_sb[(0, 1)], g_dram[(0, 1)][:, :, :])
                nc.scalar.dma_start(g_sb[(1, 0)], g_dram[(1, 0)][:, :, :])
                nc.scalar.dma_start(g_sb[(1, 1)], g_dram[(1, 1)][:, :, :])
                fetch_w1(0)
                fetch_w1(1)
                nc.sync.dma_start(b1_sb, b1p[:, :, :])
                nc.sync.dma_start(w2_sb, w2p[:, :, :])
                nc.sync.dma_start(b2_sb, b2p[:, :])

            # PE warmup: burn the p-state ramp on dummy matmuls while the
            # first ge/w1 DMAs are in flight.
            warm_sb = cpool.tile([P, 512], BF16)
            nc.vector.memset(warm_sb, 0.0)
            pwarm = psw.tile([P, 512], F32)
            for _ in range(13):
                nc.tensor.matmul(
                    pwarm, warm_sb[:, :P], warm_sb, start=True, stop=True
                )

            out_sb = opool.tile([1, ct], F32)

            # Batch schedule: each batch is (s, fetch_i, [(ci, m, first,
            # last)]) — `first`/`last` bracket that chunk's L1/L2 PSUM
            # accumulation group. Slot 0 defers chunk B by B_DELAY m-steps.
            sched = []
            fu = 2  # w1 prefetch pointer (2 already issued pre-loop)
            for s in range(SLOTS):
                bd = B_DELAY if s == 0 else 0
                for m in range(8):
                    ent = [(0, m, m == 0, m == 7)]
                    if m >= bd:
                        ent.append((1, m, m == bd, bd == 0 and m == 7))
                    sched.append((s, fu, ent))
                    fu += 1
                for m in range(bd):
                    sched.append(
                        (s, None, [(1, m, bd == 8 and m == 0, m == bd - 1)])
                    )

            pending = []  # [(s, p2s, [(ci, m, h, first, last)])]
            p2cur = None
            cur_s = None

            def emit_l2(pend):
                """Second-layer matmuls one batch behind, so the PE never
                waits on the Scalar engine's ReLU."""
                s_, p2s_, ents = pend
                for ci, m_, hsb, first, last in ents:
                    nc.tensor.matmul(
                        p2s_[ci], w2_sb[:, m_, s_ * 4:s_ * 4 + 4], hsb,
                        start=first, stop=last,
                    )
                    if last:
                        off, ln = chunks[s_][ci]
                        nc.vector.tensor_scalar_add(
                            out_sb[:, off:off + ln], p2s_[ci][0:1, :],
                            b2_sb[:, s_:s_ + 1],
                        )
                        nc.sync.dma_start(
                            out[:, off:off + ln], out_sb[:, off:off + ln]
                        )

            for s, fetch_i, ents in sched:
                if s != cur_s:
                    cur_s = s
                    p2cur = [
                        ps2.tile([4, ln], F32, tag=f"p2_{ci}", name=f"p2_{ci}")
                        for ci, (off, ln) in enumerate(chunks[s])
                    ]
                if fetch_i is not None and fetch_i < 16:
                    fetch_w1(fetch_i)
                m = ents[0][1]
                w1t = w1_tiles[(s, m)]
                p1s = {}
                for ci, m_, first, last in ents:
                    ln = chunks[s][ci][1]
                    p1s[ci] = ps1.tile(
                        [P, ln], F32, tag=f"p1_{ci}", name=f"p1_{ci}"
                    )
                for k in range(8):
                    for ci, m_, first, last in ents:
                        nc.tensor.matmul(
                            p1s[ci], w1t[:, k], g_sb[(s, ci)][:, k, :],
                            start=(k == 0), stop=(k == 7),
                        )
                # flush L2 in pairs: batching two m-steps' L2 matmuls into
                # one stationary-switch window halves the w1<->w2 weight
                # buffer switches (each costs a ~90ns pipeline bubble)
                if len(pending) == 8:
                    for pend in pending:
                        emit_l2(pend)
                    pending = []
                pents = []
                for ci, m_, first, last in ents:
                    ln = chunks[s][ci][1]
                    hsb = hpool.tile([P, ln], BF16, tag=f"h_{ci}")
                    nc.scalar.activation(
                        hsb, p1s[ci],
                        mybir.ActivationFunctionType.Relu,
                        bias=b1_sb[:, m_, s:s + 1],
                    )
                    pents.append((ci, m_, hsb, first, last))
                pending.append((s, p2cur, pents))
            for pend in pending:
                emit_l2(pend)
    nc.finalize()
    return nc


def _route_host(emb, rw, rb):
    logits = emb.astype(np.float32) @ rw.astype(np.float32) + rb.astype(np.float32)
    i1 = np.argmax(logits, axis=1)
    l1 = logits[np.arange(B), i1]
    l2m = logits.copy()
    l2m[np.arange(B), i1] = -np.inf
    i2 = np.argmax(l2m, axis=1)
    l2 = l2m[np.arange(B), i2]
    d = np.exp(l2 - l1)
    w1 = (1.0 / (1.0 + d)).astype(np.float32)
    w2 = (1.0 - w1).astype(np.float32)
    comb = np.zeros((B, E), np.float32)
    comb[np.arange(B), i1] = w1
    comb[np.arange(B), i2] = w2
    return comb


def kernel(embeddings, router_w, router_b, w1, b1, w2, b2):
    emb = np.ascontiguousarray(np.asarray(embeddings, dtype=np.float32))
    rw = np.asarray(router_w, np.float32)
    rb = np.asarray(router_b, np.float32)
    w1 = np.asarray(w1, np.float32)
    b1 = np.asarray(b1, np.float32)
    w2 = np.asarray(w2, np.float32)
    b2 = np.asarray(b2, np.float32)

    comb = _route_host(emb, rw, rb)
    counts = (comb > 0).sum(axis=0)

    # Balanced pairing: i-th largest with i-th smallest expert per core.
    order = np.argsort(counts)
    slot_experts = [  # [slot][core] -> expert id
        [int(order[E - 1 - c]) for c in range(NCORES)],
        [int(order[c]) for c in range(NCORES)],
    ]
    pad = lambda n: max(8, -(-int(n) // 4) * 4)
    c0 = pad(max(counts[e] for e in slot_experts[0]))
    c1 = pad(max(counts[e] for e in slot_experts[1]))
    ct = c0 + c1
    key = (c0, c1)
    if key not in _CACHE:
        _CACHE[key] = _build(c0, c1)
    nc = _CACHE[key]
    chunks = _chunks(c0, c1)

    emb_bf = emb.astype(BF)
    in_maps = []
    toks = []
    for c in range(NCORES):
        m = {}
        ctoks = []
        for s, cs in ((0, c0), (1, c1)):
            e = slot_experts[s][c]
            ids = np.nonzero(comb[:, e] > 0)[0]
            ctoks.append(ids)
            geT = np.zeros((P, 8, cs), BF)
            n = len(ids)
            # [n, 1024] -> [128(kp), 8(k), n]
            geT[:, :, :n] = emb_bf[ids].reshape(n, 8, P).transpose(2, 1, 0)
            for ci, (off, ln) in enumerate(chunks[s]):
                rel = off - (0 if s == 0 else c0)
                m[f"g{s}{ci}"] = np.ascontiguousarray(geT[:, :, rel:rel + ln])
        es = [slot_experts[s][c] for s in range(SLOTS)]
        # w1[e]: [h_in(k,kp), h_out(m,mp)] -> [m, kp, k, mp]
        m["w1p"] = np.ascontiguousarray(
            w1[es].reshape(SLOTS, 8, P, 8, P).transpose(0, 3, 2, 1, 4)
        ).astype(BF)
        m["b1p"] = np.ascontiguousarray(
            b1[es].reshape(SLOTS, 8, P).transpose(2, 1, 0)
        )
        m["w2p"] = np.ascontiguousarray(
            np.repeat(
                w2[es, :, 0].reshape(SLOTS, 8, P).transpose(2, 1, 0), 4, axis=2
            )
        ).astype(BF)
        m["b2p"] = np.ascontiguousarray(b2[es, 0].reshape(1, SLOTS))
        toks.append(ctoks)
        in_maps.append(m)

    res = run_bass_kernel_spmd(nc, in_maps, core_ids=list(range(NCORES)))

    out = np.zeros((B,), np.float32)
    for c in range(NCORES):
        o = res.results[c]["out"][0]  # [ct]
        for s, off in ((0, 0), (1, c0)):
            e = slot_experts[s][c]
            ids = toks[c][s]
            out[ids] += comb[ids, e] * o[off:off + len(ids)]
    return out.reshape(B, 1)
